# revision 22
# baseline (speedup 1.0000x reference)
"""Trainium2 Bass kernel for BlurredNoise: 128-filter 1D conv (K=5000) over
16 noise sequences, scaled per-filter.

Math: out[s, b, t] = sum_k noise[s, t+k] * F[b, k] * scale[b]
  s in [0,16) (= batch 2 x 8 noise channels), b in [0,128), t in [0,4096).

Mapping: data-parallel over the 16 sequences, 2 per NeuronCore. On each
core the conv is 40 accumulating 128x128x512 matmuls per output tile:
  k = 128*j + i,  lhsT_j[i, b] = F[b, 128j+i]*scale[b]  (prepped on host),
  rhs_j[i, t]    = X[t + 128j + i]   (slice of a Toeplitz band, host-built).
Operands are bf16 (fp32 PSUM accumulation): full PE rate with FWL weight
loads, measured rel-l2 error ~3e-3. Per-core: 640 matmuls at the 216 ns
warm pitch, input streaming double-ring'd and need-ordered so the PE
stream is gap-free; dead matmuls on a memset tile warm the HAM clock
gate during the first DMA's flight.
"""

import numpy as np
import ml_dtypes

import concourse.bacc as bacc
import concourse.mybir as mybir
from concourse.tile import TileContext
from concourse.bass_utils import run_bass_kernel_spmd

N_CORES = 8
BATCH = 2
NOISE_CH = 8
N_SEQ = BATCH * NOISE_CH          # 16
SEQ_PER_CORE = N_SEQ // N_CORES   # 2
T_IN = 9095
K_TAPS = 5000
T_OUT = 4096                      # T_IN - K_TAPS + 1
NJ = 40                           # ceil(5000/128)
K_PAD = NJ * 128                  # 5120
N_FILT = 128
NT = T_OUT // 512                 # 8 output tiles of 512
TAU = (NT - 1) * 512 + 512 + (NJ - 1) * 128   # 9088 Toeplitz band width
X_PAD = TAU + 128                 # 9216 >= 127 + 9087 + 1

_compiled_nc = None


def _build():
    nc = bacc.Bacc(name="blurred_noise")
    f32 = mybir.dt.float32
    bf16 = mybir.dt.bfloat16

    xt = nc.dram_tensor("xt", [SEQ_PER_CORE, 128, TAU], bf16, kind="ExternalInput")
    w = nc.dram_tensor("w", [128, K_PAD], bf16, kind="ExternalInput")
    out = nc.dram_tensor("out", [SEQ_PER_CORE, 128, T_OUT], f32, kind="ExternalOutput")

    with TileContext(nc) as tc:
        with (
            tc.tile_pool(name="wpool", bufs=1) as wp,
            tc.tile_pool(name="xpool", bufs=2) as xp,
            tc.tile_pool(name="opool", bufs=4) as op,
            tc.tile_pool(name="psum", bufs=8, space="PSUM") as pp,
        ):
            wt = wp.tile([128, K_PAD], bf16)
            # Chunked loads so the first matmuls only gate on the first slabs
            # (Tile tracks RAW deps at byte-range granularity). The head
            # chunks are small so the PE pipeline starts ~7us in; later
            # chunks are big for DMA efficiency.
            xtiles = []
            for s in range(SEQ_PER_CORE):
                xs = xp.tile([128, TAU], bf16, name=f"xs{s}")
                xtiles.append(xs)
            # All input streaming goes on the single Sync HWDGE ring, in
            # exact need-order: the ring drains FIFO, so earlier transfers
            # get full HBM bandwidth — this is the prioritization mechanism.
            # (Round-robin across two rings let background bulk steal ~half
            # the bandwidth from the critical head chunks.)
            # Weights stream on the Scalar HWDGE ring (small, finishes early;
            # later it only carries the output writes); x bands stream on the
            # Sync ring in need-order — each ring drains FIFO so earlier
            # transfers get priority, and the two head transfers overlap.
            for c0, c1 in ((0, 1536), (1536, 2560), (2560, 3584), (3584, 4608), (4608, K_PAD)):
                nc.scalar.dma_start(out=wt[:, c0:c1], in_=w[:, c0:c1])
            xloads = [
                (xtiles[0], xt[0], 0, 512),
                (xtiles[0], xt[0], 512, 1536),
                (xtiles[0], xt[0], 1536, 3072),
                (xtiles[0], xt[0], 3072, 4608),
                (xtiles[0], xt[0], 4608, 6144),
                (xtiles[0], xt[0], 6144, 7040),
                (xtiles[0], xt[0], 7040, TAU),
                (xtiles[1], xt[1], 0, 2272),
                (xtiles[1], xt[1], 2272, 4544),
                (xtiles[1], xt[1], 4544, 6816),
                (xtiles[1], xt[1], 6816, TAU),
            ]
            for dst, src, c0, c1 in xloads:
                nc.sync.dma_start(out=dst[:, c0:c1], in_=src[:, c0:c1])
            # Warm the PE HAM clock-gate while the first x slabs stream in:
            # dead matmuls on a memset tile — no DMA dependency, so the PE
            # starts right after the preamble and is at 2.4 GHz by the time
            # the first real operands land.
            wsrc = op.tile([128, 256], bf16, name="wsrc")
            nc.gpsimd.memset(wsrc[:], 0.0)
            warm = pp.tile([128, 512], f32, name="warm", tag="acc")
            for i in range(44):
                nc.tensor.matmul(
                    warm[:, 0:128], wsrc[:, 0:128], wsrc[:, 128:256],
                    start=True, stop=True,
                )
            # s1's tail groups are 2-wide so the final PSUM->SBUF copy chain
            # after the very last matmul is short.
            groupings = {0: [(0, 4), (4, 4)], 1: [(0, 4), (4, 2), (6, 2)]}
            for s in range(SEQ_PER_CORE):
                xs = xtiles[s]
                for gi, (tbase, glen) in enumerate(groupings[s]):
                    ptiles = [
                        pp.tile([128, 512], f32, name=f"acc_{s}_{gi}_{i}", tag="acc")
                        for i in range(glen)
                    ]
                    for j in range(NJ):
                        lhsT = wt[:, j * 128:(j + 1) * 128]
                        for tt in range(glen):
                            col0 = (tbase + tt) * 512 + j * 128
                            nc.tensor.matmul(
                                ptiles[tt][:],
                                lhsT,
                                xs[:, col0:col0 + 512],
                                start=(j == 0),
                                stop=(j == NJ - 1),
                            )
                    for tt in range(glen):
                        t0 = (tbase + tt) * 512
                        ot = op.tile([128, 512], f32)
                        nc.vector.tensor_copy(ot[:], ptiles[tt][:])
                        nc.scalar.dma_start(out=out[s][:, t0:t0 + 512], in_=ot[:])
    nc.compile()
    return nc


def _get_nc():
    global _compiled_nc
    if _compiled_nc is None:
        _compiled_nc = _build()
    return _compiled_nc


def _prep_inputs(noise, blur_filters, output_scale):
    noise = np.ascontiguousarray(np.asarray(noise, dtype=np.float32))
    F = np.asarray(blur_filters, dtype=np.float32)
    scale = np.asarray(output_scale, dtype=np.float32).reshape(N_FILT)

    # Fold the per-filter output scale into the filters, zero-pad taps to 5120,
    # and lay out as W[i, 128j + b] = F[b, 128j + i] (contraction dim on
    # partitions, filter dim on the matmul free axis).
    gain = 1.0 + 1.0 * (scale - 1.0)
    Fp = np.zeros((N_FILT, K_PAD), dtype=np.float32)
    Fp[:, :K_TAPS] = F * gain[:, None]
    W = np.ascontiguousarray(
        Fp.reshape(N_FILT, NJ, 128).transpose(2, 1, 0).reshape(128, NJ * 128)
    ).astype(ml_dtypes.bfloat16)

    # Toeplitz band per sequence: xt[s, i, tau] = X[s, i + tau].
    Xflat = np.zeros((N_SEQ, X_PAD), dtype=ml_dtypes.bfloat16)
    Xflat[:, :T_IN] = noise.reshape(N_SEQ, T_IN)
    sv = np.lib.stride_tricks.sliding_window_view(Xflat, TAU, axis=1)  # (16, 129, TAU)
    in_maps = []
    for c in range(N_CORES):
        xt = np.ascontiguousarray(
            sv[c * SEQ_PER_CORE:(c + 1) * SEQ_PER_CORE, :128, :]
        )  # (2, 128, TAU)
        in_maps.append({"xt": xt, "w": W})
    return in_maps


def _run(noise, blur_filters, output_scale, trace=False, tmpdir=None):
    in_maps = _prep_inputs(noise, blur_filters, output_scale)
    nc = _get_nc()
    res = run_bass_kernel_spmd(
        nc, in_maps, list(range(N_CORES)), trace=trace, tmpdir=tmpdir
    )
    outs = np.stack([res.results[c]["out"] for c in range(N_CORES)])  # (8, 2, 128, 4096)
    full = outs.reshape(BATCH, NOISE_CH, N_FILT, T_OUT).reshape(BATCH, NOISE_CH * N_FILT, T_OUT)
    return np.ascontiguousarray(full), res


def kernel(noise, blur_filters, output_scale):
    full, _ = _run(noise, blur_filters, output_scale)
    return full


# revision 23
# speedup vs baseline: 1.0135x; 1.0135x over previous
"""Trainium2 Bass kernel for BlurredNoise: 128-filter 1D conv (K=5000) over
16 noise sequences, scaled per-filter.

Math: out[s, b, t] = sum_k noise[s, t+k] * F[b, k] * scale[b]
  s in [0,16) (= batch 2 x 8 noise channels), b in [0,128), t in [0,4096).

Mapping: data-parallel over the 16 sequences, 2 per NeuronCore. On each
core the conv is 40 accumulating 128x128x512 matmuls per output tile:
  k = 128*j + i,  lhsT_j[i, b] = F[b, 128j+i]*scale[b]  (prepped on host),
  rhs_j[i, t]    = X[t + 128j + i]   (slice of a Toeplitz band, host-built).
Operands are bf16 (fp32 PSUM accumulation): full PE rate with FWL weight
loads, measured rel-l2 error ~3e-3. Per-core: 640 matmuls at the 216 ns
warm pitch, input streaming double-ring'd and need-ordered so the PE
stream is gap-free; dead matmuls on a memset tile warm the HAM clock
gate during the first DMA's flight.
"""

import numpy as np
import ml_dtypes

import concourse.bacc as bacc
import concourse.mybir as mybir
from concourse.tile import TileContext
from concourse.bass_utils import run_bass_kernel_spmd

N_CORES = 8
BATCH = 2
NOISE_CH = 8
N_SEQ = BATCH * NOISE_CH          # 16
SEQ_PER_CORE = N_SEQ // N_CORES   # 2
T_IN = 9095
K_TAPS = 5000
T_OUT = 4096                      # T_IN - K_TAPS + 1
NJ = 40                           # ceil(5000/128)
K_PAD = NJ * 128                  # 5120
N_FILT = 128
NT = T_OUT // 512                 # 8 output tiles of 512
TAU = (NT - 1) * 512 + 512 + (NJ - 1) * 128   # 9088 Toeplitz band width
X_PAD = TAU + 128                 # 9216 >= 127 + 9087 + 1

_compiled_nc = None


def _build():
    nc = bacc.Bacc(name="blurred_noise")
    f32 = mybir.dt.float32
    bf16 = mybir.dt.bfloat16

    xt = nc.dram_tensor("xt", [SEQ_PER_CORE, 128, TAU], bf16, kind="ExternalInput")
    w = nc.dram_tensor("w", [128, K_PAD], bf16, kind="ExternalInput")
    out = nc.dram_tensor("out", [SEQ_PER_CORE, 128, T_OUT], f32, kind="ExternalOutput")

    with TileContext(nc) as tc:
        with (
            tc.tile_pool(name="wpool", bufs=1) as wp,
            tc.tile_pool(name="xpool", bufs=2) as xp,
            tc.tile_pool(name="opool", bufs=4) as op,
            tc.tile_pool(name="psum", bufs=8, space="PSUM") as pp,
        ):
            wt = wp.tile([128, K_PAD], bf16)
            # Chunked loads so the first matmuls only gate on the first slabs
            # (Tile tracks RAW deps at byte-range granularity). The head
            # chunks are small so the PE pipeline starts ~7us in; later
            # chunks are big for DMA efficiency.
            xtiles = []
            for s in range(SEQ_PER_CORE):
                xs = xp.tile([128, TAU], bf16, name=f"xs{s}")
                xtiles.append(xs)
            # All input streaming goes on the single Sync HWDGE ring, in
            # exact need-order: the ring drains FIFO, so earlier transfers
            # get full HBM bandwidth — this is the prioritization mechanism.
            # (Round-robin across two rings let background bulk steal ~half
            # the bandwidth from the critical head chunks.)
            # Weights stream on the Scalar HWDGE ring (small, finishes early;
            # later it only carries the output writes); x bands stream on the
            # Sync ring in need-order — each ring drains FIFO so earlier
            # transfers get priority, and the two head transfers overlap.
            for c0, c1 in ((0, 1536), (1536, 2560), (2560, 3584), (3584, 4608), (4608, K_PAD)):
                nc.scalar.dma_start(out=wt[:, c0:c1], in_=w[:, c0:c1])
            xloads = [
                (xtiles[0], xt[0], 0, 1536),
                (xtiles[0], xt[0], 1536, 3072),
                (xtiles[0], xt[0], 3072, 4608),
                (xtiles[0], xt[0], 4608, 6144),
                (xtiles[0], xt[0], 6144, 7040),
                (xtiles[0], xt[0], 7040, TAU),
                (xtiles[1], xt[1], 0, 2272),
                (xtiles[1], xt[1], 2272, 4544),
                (xtiles[1], xt[1], 4544, 6816),
                (xtiles[1], xt[1], 6816, TAU),
            ]
            for dst, src, c0, c1 in xloads:
                nc.sync.dma_start(out=dst[:, c0:c1], in_=src[:, c0:c1])
            # Warm the PE HAM clock-gate while the first x slabs stream in:
            # dead matmuls on a memset tile — no DMA dependency, so the PE
            # starts right after the preamble and is at 2.4 GHz by the time
            # the first real operands land.
            wsrc = op.tile([128, 256], bf16, name="wsrc")
            nc.gpsimd.memset(wsrc[:], 0.0)
            warm = pp.tile([128, 512], f32, name="warm", tag="acc")
            for i in range(44):
                nc.tensor.matmul(
                    warm[:, 0:128], wsrc[:, 0:128], wsrc[:, 128:256],
                    start=True, stop=True,
                )
            # s1's tail groups are 2-wide so the final PSUM->SBUF copy chain
            # after the very last matmul is short.
            groupings = {0: [(0, 4), (4, 4)], 1: [(0, 4), (4, 2), (6, 2)]}
            for s in range(SEQ_PER_CORE):
                xs = xtiles[s]
                for gi, (tbase, glen) in enumerate(groupings[s]):
                    ptiles = [
                        pp.tile([128, 512], f32, name=f"acc_{s}_{gi}_{i}", tag="acc")
                        for i in range(glen)
                    ]
                    for j in range(NJ):
                        lhsT = wt[:, j * 128:(j + 1) * 128]
                        for tt in range(glen):
                            col0 = (tbase + tt) * 512 + j * 128
                            nc.tensor.matmul(
                                ptiles[tt][:],
                                lhsT,
                                xs[:, col0:col0 + 512],
                                start=(j == 0),
                                stop=(j == NJ - 1),
                            )
                    for tt in range(glen):
                        t0 = (tbase + tt) * 512
                        ot = op.tile([128, 512], f32)
                        nc.vector.tensor_copy(ot[:], ptiles[tt][:])
                        nc.scalar.dma_start(out=out[s][:, t0:t0 + 512], in_=ot[:])
    nc.compile()
    return nc


def _get_nc():
    global _compiled_nc
    if _compiled_nc is None:
        _compiled_nc = _build()
    return _compiled_nc


def _prep_inputs(noise, blur_filters, output_scale):
    noise = np.ascontiguousarray(np.asarray(noise, dtype=np.float32))
    F = np.asarray(blur_filters, dtype=np.float32)
    scale = np.asarray(output_scale, dtype=np.float32).reshape(N_FILT)

    # Fold the per-filter output scale into the filters, zero-pad taps to 5120,
    # and lay out as W[i, 128j + b] = F[b, 128j + i] (contraction dim on
    # partitions, filter dim on the matmul free axis).
    gain = 1.0 + 1.0 * (scale - 1.0)
    Fp = np.zeros((N_FILT, K_PAD), dtype=np.float32)
    Fp[:, :K_TAPS] = F * gain[:, None]
    W = np.ascontiguousarray(
        Fp.reshape(N_FILT, NJ, 128).transpose(2, 1, 0).reshape(128, NJ * 128)
    ).astype(ml_dtypes.bfloat16)

    # Toeplitz band per sequence: xt[s, i, tau] = X[s, i + tau].
    Xflat = np.zeros((N_SEQ, X_PAD), dtype=ml_dtypes.bfloat16)
    Xflat[:, :T_IN] = noise.reshape(N_SEQ, T_IN)
    sv = np.lib.stride_tricks.sliding_window_view(Xflat, TAU, axis=1)  # (16, 129, TAU)
    in_maps = []
    for c in range(N_CORES):
        xt = np.ascontiguousarray(
            sv[c * SEQ_PER_CORE:(c + 1) * SEQ_PER_CORE, :128, :]
        )  # (2, 128, TAU)
        in_maps.append({"xt": xt, "w": W})
    return in_maps


def _run(noise, blur_filters, output_scale, trace=False, tmpdir=None):
    in_maps = _prep_inputs(noise, blur_filters, output_scale)
    nc = _get_nc()
    res = run_bass_kernel_spmd(
        nc, in_maps, list(range(N_CORES)), trace=trace, tmpdir=tmpdir
    )
    outs = np.stack([res.results[c]["out"] for c in range(N_CORES)])  # (8, 2, 128, 4096)
    full = outs.reshape(BATCH, NOISE_CH, N_FILT, T_OUT).reshape(BATCH, NOISE_CH * N_FILT, T_OUT)
    return np.ascontiguousarray(full), res


def kernel(noise, blur_filters, output_scale):
    full, _ = _run(noise, blur_filters, output_scale)
    return full


# revision 24
# speedup vs baseline: 1.0146x; 1.0011x over previous
"""Trainium2 Bass kernel for BlurredNoise: 128-filter 1D conv (K=5000) over
16 noise sequences, scaled per-filter.

Math: out[s, b, t] = sum_k noise[s, t+k] * F[b, k] * scale[b]
  s in [0,16) (= batch 2 x 8 noise channels), b in [0,128), t in [0,4096).

Mapping: data-parallel over the 16 sequences, 2 per NeuronCore. On each
core the conv is 40 accumulating 128x128x512 matmuls per output tile:
  k = 128*j + i,  lhsT_j[i, b] = F[b, 128j+i]*scale[b]  (prepped on host),
  rhs_j[i, t]    = X[t + 128j + i]   (slice of a Toeplitz band, host-built).
Operands are bf16 (fp32 PSUM accumulation): full PE rate with FWL weight
loads, measured rel-l2 error ~3e-3. Per-core: 640 matmuls at the 216 ns
warm pitch, input streaming double-ring'd and need-ordered so the PE
stream is gap-free; dead matmuls on a memset tile warm the HAM clock
gate during the first DMA's flight.
"""

import numpy as np
import ml_dtypes

import concourse.bacc as bacc
import concourse.mybir as mybir
from concourse.tile import TileContext
from concourse.bass_utils import run_bass_kernel_spmd

N_CORES = 8
BATCH = 2
NOISE_CH = 8
N_SEQ = BATCH * NOISE_CH          # 16
SEQ_PER_CORE = N_SEQ // N_CORES   # 2
T_IN = 9095
K_TAPS = 5000
T_OUT = 4096                      # T_IN - K_TAPS + 1
NJ = 40                           # ceil(5000/128)
K_PAD = NJ * 128                  # 5120
N_FILT = 128
NT = T_OUT // 512                 # 8 output tiles of 512
TAU = (NT - 1) * 512 + 512 + (NJ - 1) * 128   # 9088 Toeplitz band width
X_PAD = TAU + 128                 # 9216 >= 127 + 9087 + 1

_compiled_nc = None


def _build():
    nc = bacc.Bacc(name="blurred_noise")
    f32 = mybir.dt.float32
    bf16 = mybir.dt.bfloat16

    xt = nc.dram_tensor("xt", [SEQ_PER_CORE, 128, TAU], bf16, kind="ExternalInput")
    w = nc.dram_tensor("w", [128, K_PAD], bf16, kind="ExternalInput")
    out = nc.dram_tensor("out", [SEQ_PER_CORE, 128, T_OUT], f32, kind="ExternalOutput")

    with TileContext(nc) as tc:
        with (
            tc.tile_pool(name="wpool", bufs=1) as wp,
            tc.tile_pool(name="xpool", bufs=2) as xp,
            tc.tile_pool(name="opool", bufs=4) as op,
            tc.tile_pool(name="psum", bufs=8, space="PSUM") as pp,
        ):
            wt = wp.tile([128, K_PAD], bf16)
            xtiles = []
            for s in range(SEQ_PER_CORE):
                xs = xp.tile([128, TAU], bf16, name=f"xs{s}")
                xtiles.append(xs)
            # Inputs stream in chunks so the first matmuls only gate on the
            # first slabs (Tile tracks RAW deps at byte-range granularity).
            # Each HWDGE ring drains FIFO, so ordering transfers by first-use
            # IS the prioritization mechanism; two rings are used so the two
            # head transfers (first weight chunk, first x band) overlap.
            # Weights go on the Scalar ring (small, finishes early; later it
            # only carries output writes), x bands on the Sync ring.
            for c0, c1 in ((0, 1536), (1536, 2560), (2560, 3584), (3584, 4608), (4608, K_PAD)):
                nc.scalar.dma_start(out=wt[:, c0:c1], in_=w[:, c0:c1])
            xloads = [
                (xtiles[0], xt[0], 0, 1536),
                (xtiles[0], xt[0], 1536, 3072),
                (xtiles[0], xt[0], 3072, 4608),
                (xtiles[0], xt[0], 4608, 6144),
                (xtiles[0], xt[0], 6144, 7040),
                (xtiles[0], xt[0], 7040, TAU),
                (xtiles[1], xt[1], 0, 2272),
                (xtiles[1], xt[1], 2272, 4544),
                (xtiles[1], xt[1], 4544, 6816),
                (xtiles[1], xt[1], 6816, TAU),
            ]
            for dst, src, c0, c1 in xloads:
                nc.sync.dma_start(out=dst[:, c0:c1], in_=src[:, c0:c1])
            # Warm the PE HAM clock-gate while the first x slabs stream in:
            # dead matmuls on a memset tile — no DMA dependency, so the PE
            # starts right after the preamble and is at 2.4 GHz by the time
            # the first real operands land.
            wsrc = op.tile([128, 256], bf16, name="wsrc")
            nc.gpsimd.memset(wsrc[:], 0.0)
            warm = pp.tile([128, 512], f32, name="warm", tag="acc")
            for i in range(44):
                nc.tensor.matmul(
                    warm[:, 0:128], wsrc[:, 0:128], wsrc[:, 128:256],
                    start=True, stop=True,
                )
            # s1's tail groups are 2-wide so the final PSUM->SBUF copy chain
            # after the very last matmul is short.
            groupings = {0: [(0, 4), (4, 4)], 1: [(0, 4), (4, 2), (6, 2)]}
            for s in range(SEQ_PER_CORE):
                xs = xtiles[s]
                for gi, (tbase, glen) in enumerate(groupings[s]):
                    ptiles = [
                        pp.tile([128, 512], f32, name=f"acc_{s}_{gi}_{i}", tag="acc")
                        for i in range(glen)
                    ]
                    for j in range(NJ):
                        lhsT = wt[:, j * 128:(j + 1) * 128]
                        for tt in range(glen):
                            col0 = (tbase + tt) * 512 + j * 128
                            nc.tensor.matmul(
                                ptiles[tt][:],
                                lhsT,
                                xs[:, col0:col0 + 512],
                                start=(j == 0),
                                stop=(j == NJ - 1),
                            )
                    for tt in range(glen):
                        t0 = (tbase + tt) * 512
                        ot = op.tile([128, 512], f32)
                        nc.vector.tensor_copy(ot[:], ptiles[tt][:])
                        nc.scalar.dma_start(out=out[s][:, t0:t0 + 512], in_=ot[:])
    nc.compile()
    return nc


def _get_nc():
    global _compiled_nc
    if _compiled_nc is None:
        _compiled_nc = _build()
    return _compiled_nc


def _prep_inputs(noise, blur_filters, output_scale):
    noise = np.ascontiguousarray(np.asarray(noise, dtype=np.float32))
    F = np.asarray(blur_filters, dtype=np.float32)
    scale = np.asarray(output_scale, dtype=np.float32).reshape(N_FILT)

    # Fold the per-filter output scale into the filters, zero-pad taps to 5120,
    # and lay out as W[i, 128j + b] = F[b, 128j + i] (contraction dim on
    # partitions, filter dim on the matmul free axis).
    gain = 1.0 + 1.0 * (scale - 1.0)
    Fp = np.zeros((N_FILT, K_PAD), dtype=np.float32)
    Fp[:, :K_TAPS] = F * gain[:, None]
    W = np.ascontiguousarray(
        Fp.reshape(N_FILT, NJ, 128).transpose(2, 1, 0).reshape(128, NJ * 128)
    ).astype(ml_dtypes.bfloat16)

    # Toeplitz band per sequence: xt[s, i, tau] = X[s, i + tau].
    Xflat = np.zeros((N_SEQ, X_PAD), dtype=ml_dtypes.bfloat16)
    Xflat[:, :T_IN] = noise.reshape(N_SEQ, T_IN)
    sv = np.lib.stride_tricks.sliding_window_view(Xflat, TAU, axis=1)  # (16, 129, TAU)
    in_maps = []
    for c in range(N_CORES):
        xt = np.ascontiguousarray(
            sv[c * SEQ_PER_CORE:(c + 1) * SEQ_PER_CORE, :128, :]
        )  # (2, 128, TAU)
        in_maps.append({"xt": xt, "w": W})
    return in_maps


def _run(noise, blur_filters, output_scale, trace=False, tmpdir=None):
    in_maps = _prep_inputs(noise, blur_filters, output_scale)
    nc = _get_nc()
    res = run_bass_kernel_spmd(
        nc, in_maps, list(range(N_CORES)), trace=trace, tmpdir=tmpdir
    )
    outs = np.stack([res.results[c]["out"] for c in range(N_CORES)])  # (8, 2, 128, 4096)
    full = outs.reshape(BATCH, NOISE_CH, N_FILT, T_OUT).reshape(BATCH, NOISE_CH * N_FILT, T_OUT)
    return np.ascontiguousarray(full), res


def kernel(noise, blur_filters, output_scale):
    full, _ = _run(noise, blur_filters, output_scale)
    return full


# revision 25
# speedup vs baseline: 1.0155x; 1.0008x over previous
"""Trainium2 Bass kernel for BlurredNoise: 128-filter 1D conv (K=5000) over
16 noise sequences, scaled per-filter.

Math: out[s, b, t] = sum_k noise[s, t+k] * F[b, k] * scale[b]
  s in [0,16) (= batch 2 x 8 noise channels), b in [0,128), t in [0,4096).

Mapping: data-parallel over the 16 sequences, 2 per NeuronCore. On each
core the conv is 40 accumulating 128x128x512 matmuls per output tile:
  k = 128*j + i,  lhsT_j[i, b] = F[b, 128j+i]*scale[b]  (prepped on host),
  rhs_j[i, t]    = X[t + 128j + i]   (slice of a Toeplitz band, host-built).
Operands are bf16 (fp32 PSUM accumulation): full PE rate with FWL weight
loads, measured rel-l2 error ~3e-3. Per-core: 640 matmuls at the 216 ns
warm pitch, input streaming double-ring'd and need-ordered so the PE
stream is gap-free; dead matmuls on a memset tile warm the HAM clock
gate during the first DMA's flight.
"""

import numpy as np
import ml_dtypes

import concourse.bacc as bacc
import concourse.mybir as mybir
from concourse.tile import TileContext
from concourse.bass_utils import run_bass_kernel_spmd

N_CORES = 8
BATCH = 2
NOISE_CH = 8
N_SEQ = BATCH * NOISE_CH          # 16
SEQ_PER_CORE = N_SEQ // N_CORES   # 2
T_IN = 9095
K_TAPS = 5000
T_OUT = 4096                      # T_IN - K_TAPS + 1
NJ = 40                           # ceil(5000/128)
K_PAD = NJ * 128                  # 5120
N_FILT = 128
NT = T_OUT // 512                 # 8 output tiles of 512
TAU = (NT - 1) * 512 + 512 + (NJ - 1) * 128   # 9088 Toeplitz band width
X_PAD = TAU + 128                 # 9216 >= 127 + 9087 + 1

_compiled_nc = None


def _build():
    nc = bacc.Bacc(name="blurred_noise")
    f32 = mybir.dt.float32
    bf16 = mybir.dt.bfloat16

    xt = nc.dram_tensor("xt", [SEQ_PER_CORE, 128, TAU], bf16, kind="ExternalInput")
    w = nc.dram_tensor("w", [128, K_PAD], bf16, kind="ExternalInput")
    out = nc.dram_tensor("out", [SEQ_PER_CORE, 128, T_OUT], f32, kind="ExternalOutput")

    with TileContext(nc) as tc:
        with (
            tc.tile_pool(name="wpool", bufs=1) as wp,
            tc.tile_pool(name="xpool", bufs=2) as xp,
            tc.tile_pool(name="opool", bufs=4) as op,
            tc.tile_pool(name="psum", bufs=8, space="PSUM") as pp,
        ):
            wt = wp.tile([128, K_PAD], bf16)
            xtiles = []
            for s in range(SEQ_PER_CORE):
                xs = xp.tile([128, TAU], bf16, name=f"xs{s}")
                xtiles.append(xs)
            # Inputs stream in chunks so the first matmuls only gate on the
            # first slabs (Tile tracks RAW deps at byte-range granularity).
            # Each HWDGE ring drains FIFO, so ordering transfers by first-use
            # IS the prioritization mechanism; two rings are used so the two
            # head transfers (first weight chunk, first x band) overlap.
            # Weights go on the Scalar ring (small, finishes early; later it
            # only carries output writes), x bands on the Sync ring.
            for c0, c1 in ((0, 1536), (1536, 2560), (2560, 3584), (3584, 4608), (4608, K_PAD)):
                nc.scalar.dma_start(out=wt[:, c0:c1], in_=w[:, c0:c1])
            xloads = [
                (xtiles[0], xt[0], 0, 1536),
                (xtiles[0], xt[0], 1536, 3072),
                (xtiles[0], xt[0], 3072, 4608),
                (xtiles[0], xt[0], 4608, 6144),
                (xtiles[0], xt[0], 6144, 7040),
                (xtiles[0], xt[0], 7040, TAU),
                (xtiles[1], xt[1], 0, 2272),
                (xtiles[1], xt[1], 2272, 4544),
                (xtiles[1], xt[1], 4544, 6816),
                (xtiles[1], xt[1], 6816, TAU),
            ]
            for dst, src, c0, c1 in xloads:
                nc.sync.dma_start(out=dst[:, c0:c1], in_=src[:, c0:c1])
            # Warm the PE HAM clock-gate while the first x slabs stream in:
            # dead matmuls on a memset tile — no DMA dependency, so the PE
            # starts right after the preamble and is at 2.4 GHz by the time
            # the first real operands land.
            wsrc = op.tile([128, 256], bf16, name="wsrc")
            nc.gpsimd.memset(wsrc[:], 0.0)
            warm = pp.tile([128, 512], f32, name="warm", tag="acc")
            for i in range(44):
                nc.tensor.matmul(
                    warm[:, 0:128], wsrc[:, 0:128], wsrc[:, 128:256],
                    start=True, stop=True,
                )
            # s1's tail groups narrow to 1-wide so the copy/DMA chain after
            # the very last matmul is as short as possible.
            groupings = {0: [(0, 4), (4, 4)], 1: [(0, 4), (4, 2), (6, 1), (7, 1)]}
            last = (SEQ_PER_CORE - 1, len(groupings[SEQ_PER_CORE - 1]) - 1)
            for s in range(SEQ_PER_CORE):
                xs = xtiles[s]
                for gi, (tbase, glen) in enumerate(groupings[s]):
                    ptiles = [
                        pp.tile([128, 512], f32, name=f"acc_{s}_{gi}_{i}", tag="acc")
                        for i in range(glen)
                    ]
                    for j in range(NJ):
                        lhsT = wt[:, j * 128:(j + 1) * 128]
                        for tt in range(glen):
                            col0 = (tbase + tt) * 512 + j * 128
                            nc.tensor.matmul(
                                ptiles[tt][:],
                                lhsT,
                                xs[:, col0:col0 + 512],
                                start=(j == 0),
                                stop=(j == NJ - 1),
                            )
                    for tt in range(glen):
                        t0 = (tbase + tt) * 512
                        ot = op.tile([128, 512], f32)
                        if (s, gi) == last and tt == glen - 1:
                            # Half-copies let the first half's DMA launch
                            # while the second half is still copying.
                            nc.vector.tensor_copy(ot[:, 0:256], ptiles[tt][:, 0:256])
                            nc.scalar.dma_start(out=out[s][:, t0:t0 + 256], in_=ot[:, 0:256])
                            nc.vector.tensor_copy(ot[:, 256:512], ptiles[tt][:, 256:512])
                            nc.scalar.dma_start(out=out[s][:, t0 + 256:t0 + 512], in_=ot[:, 256:512])
                        else:
                            nc.vector.tensor_copy(ot[:], ptiles[tt][:])
                            nc.scalar.dma_start(out=out[s][:, t0:t0 + 512], in_=ot[:])
    nc.compile()
    return nc


def _get_nc():
    global _compiled_nc
    if _compiled_nc is None:
        _compiled_nc = _build()
    return _compiled_nc


def _prep_inputs(noise, blur_filters, output_scale):
    noise = np.ascontiguousarray(np.asarray(noise, dtype=np.float32))
    F = np.asarray(blur_filters, dtype=np.float32)
    scale = np.asarray(output_scale, dtype=np.float32).reshape(N_FILT)

    # Fold the per-filter output scale into the filters, zero-pad taps to 5120,
    # and lay out as W[i, 128j + b] = F[b, 128j + i] (contraction dim on
    # partitions, filter dim on the matmul free axis).
    gain = 1.0 + 1.0 * (scale - 1.0)
    Fp = np.zeros((N_FILT, K_PAD), dtype=np.float32)
    Fp[:, :K_TAPS] = F * gain[:, None]
    W = np.ascontiguousarray(
        Fp.reshape(N_FILT, NJ, 128).transpose(2, 1, 0).reshape(128, NJ * 128)
    ).astype(ml_dtypes.bfloat16)

    # Toeplitz band per sequence: xt[s, i, tau] = X[s, i + tau].
    Xflat = np.zeros((N_SEQ, X_PAD), dtype=ml_dtypes.bfloat16)
    Xflat[:, :T_IN] = noise.reshape(N_SEQ, T_IN)
    sv = np.lib.stride_tricks.sliding_window_view(Xflat, TAU, axis=1)  # (16, 129, TAU)
    in_maps = []
    for c in range(N_CORES):
        xt = np.ascontiguousarray(
            sv[c * SEQ_PER_CORE:(c + 1) * SEQ_PER_CORE, :128, :]
        )  # (2, 128, TAU)
        in_maps.append({"xt": xt, "w": W})
    return in_maps


def _run(noise, blur_filters, output_scale, trace=False, tmpdir=None):
    in_maps = _prep_inputs(noise, blur_filters, output_scale)
    nc = _get_nc()
    res = run_bass_kernel_spmd(
        nc, in_maps, list(range(N_CORES)), trace=trace, tmpdir=tmpdir
    )
    outs = np.stack([res.results[c]["out"] for c in range(N_CORES)])  # (8, 2, 128, 4096)
    full = outs.reshape(BATCH, NOISE_CH, N_FILT, T_OUT).reshape(BATCH, NOISE_CH * N_FILT, T_OUT)
    return np.ascontiguousarray(full), res


def kernel(noise, blur_filters, output_scale):
    full, _ = _run(noise, blur_filters, output_scale)
    return full
